# revision 17
# baseline (speedup 1.0000x reference)
"""HGCN decoder kernel for Trainium2, 8-core data-parallel SPMD.

Math: the reference's per-layer hyperbolic sandwich
    h = proj(expmap0(relu(agg)));  next-layer t = logmap0(h)
collapses analytically to a norm clip:  t = r * min(1, Z/||r||) with
Z = artanh(MAX_NORM), because logmap0(proj(expmap0(v))) == v when
tanh(||v||) <= MAX_NORM and == v * Z/||v|| otherwise.  The input stage
keeps the genuine artanh scaling (points start inside the ball).

Layout: activations live in "s-layout" tiles [128, 256]:
    ts[p, c*128 + j] = t[node j, dim c*128 + p]   (c = dim-chunk 0/1)
so the linear (contract over d) uses lhsT = ts chunks directly, and the
adjacency aggregation (contract over n_in) uses lhsT = u (the linear's
natural [n, d'] PSUM output) with rhs = adj^T (pre-transposed on host).
The loop closes with zero on-chip transposes.
"""

from contextlib import ExitStack

import numpy as np

import concourse.bacc as bacc
import concourse.bass as bass
import concourse.tile as tile
from concourse import mybir
from concourse.bass_utils import run_bass_kernel_spmd

# problem dims (hardcoded per contract)
B, N, D, F, L = 512, 128, 256, 16, 3
NCORES = 8
BPC = B // NCORES  # 64 batches per core
BT = 4  # batches per scale-chain group
EPS = float(np.float32(1e-7))
MAX_NORM = float(np.float32(1.0 - 1e-5))
# clip radius: artanh(MAX_NORM) evaluated like the reference would (fp32 input)
Z = float(np.float32(np.arctanh(np.float64(np.float32(1.0 - 1e-5)))))

F32 = mybir.dt.float32
F32R = mybir.dt.float32r
AF = mybir.ActivationFunctionType


def _build(has_bias: bool, has_bout: bool, bpc: int = BPC) -> bass.Bass:
    nc = bacc.Bacc()

    xT_d = nc.dram_tensor("xT", [bpc, 2, 128, N], F32R, kind="ExternalInput")
    adjT_d = nc.dram_tensor("adjT", [bpc, N, N], F32, kind="ExternalInput")
    mask_d = nc.dram_tensor("mask", [bpc, N, 1], F32, kind="ExternalInput")
    W_d = nc.dram_tensor("Ws", [L, D, D], F32R, kind="ExternalInput")
    Wout_d = nc.dram_tensor("Wout", [D, F], F32R, kind="ExternalInput")
    if has_bias:
        bs_d = nc.dram_tensor("bs", [L, 1, D], F32, kind="ExternalInput")
    if has_bout:
        bout_d = nc.dram_tensor("bout", [1, F], F32, kind="ExternalInput")
    out_d = nc.dram_tensor("out", [bpc, N, F], F32, kind="ExternalOutput")

    with tile.TileContext(nc) as tc, ExitStack() as ctx:
        singles = ctx.enter_context(tc.tile_pool(name="singles", bufs=1))
        p_x = ctx.enter_context(tc.tile_pool(name="xs", bufs=2 * BT + 2))
        p_adj = ctx.enter_context(tc.tile_pool(name="adj", bufs=2 * BT + 2))
        p_u = ctx.enter_context(tc.tile_pool(name="u", bufs=3))
        p_r = ctx.enter_context(tc.tile_pool(name="r", bufs=BT + 2))
        p_sq = ctx.enter_context(tc.tile_pool(name="sq", bufs=5))
        p_sc = ctx.enter_context(tc.tile_pool(name="sc", bufs=3))
        p_tmp = ctx.enter_context(tc.tile_pool(name="tmp", bufs=6))
        p_out = ctx.enter_context(tc.tile_pool(name="ho", bufs=4))
        pp_u = ctx.enter_context(tc.tile_pool(name="ppu", bufs=2, space="PSUM"))
        pp_o2 = ctx.enter_context(tc.tile_pool(name="ppo2", bufs=2, space="PSUM"))
        pp_n = ctx.enter_context(tc.tile_pool(name="ppn", bufs=2, space="PSUM"))
        pp_h = ctx.enter_context(tc.tile_pool(name="pph", bufs=2, space="PSUM"))

        # weights resident in SBUF: layer i, k-chunk c at cols (i*2+c)*256
        W_sb = singles.tile([128, L * 2 * D], F32R)
        for i in range(L):
            for c in range(2):
                nc.sync.dma_start(
                    out=W_sb[:, (i * 2 + c) * D : (i * 2 + c + 1) * D],
                    in_=W_d[i, c * 128 : (c + 1) * 128, :],
                )
        Wout_sb = singles.tile([128, 2 * F], F32R)
        for c in range(2):
            nc.sync.dma_start(
                out=Wout_sb[:, c * F : (c + 1) * F],
                in_=Wout_d[c * 128 : (c + 1) * 128, :],
            )
        ones_col = singles.tile([128, 1], F32)
        nc.vector.memset(ones_col, 1.0)
        # all node masks resident: column b = mask for batch b  [128, bpc]
        mask_sb = singles.tile([128, bpc], F32)
        nc.sync.dma_start(out=mask_sb, in_=mask_d.rearrange("b n one -> n (b one)"))
        if has_bias:
            ones_row = singles.tile([1, 128], F32)
            nc.vector.memset(ones_row, 1.0)
            bs_sb = singles.tile([1, L * D], F32)
            for i in range(L):
                nc.sync.dma_start(out=bs_sb[:, i * D : (i + 1) * D], in_=bs_d[i])
        if has_bout:
            if not has_bias:
                ones_row = singles.tile([1, 128], F32)
                nc.vector.memset(ones_row, 1.0)
            bout_sb = singles.tile([1, F], F32)
            nc.sync.dma_start(out=bout_sb, in_=bout_d)

        def norm_mm(nsq_col, sq_tile):
            """nsq_col[n,1] = sum_d sq_tile (s-layout) via ones-rhs matmuls."""
            for c in range(2):
                nc.tensor.matmul(
                    nsq_col,
                    sq_tile[:, c * 128 : (c + 1) * 128],
                    ones_col,
                    start=(c == 0),
                    stop=(c == 1),
                )

        def clip_chain(nsq_ps):
            """sc = min(1, Z / max(sqrt(nsq), EPS)) on [128, BT]."""
            n2 = p_tmp.tile([128, BT], F32, tag="t0")
            nc.vector.tensor_scalar_max(n2, nsq_ps, EPS * EPS)
            nn = p_tmp.tile([128, BT], F32, tag="t1")
            nc.scalar.activation(nn, n2, AF.Sqrt)
            rn = p_tmp.tile([128, BT], F32, tag="t2")
            nc.vector.reciprocal(rn, nn)
            sc = p_sc.tile([128, BT], F32)
            nc.vector.tensor_scalar(sc, rn, Z, 1.0, mybir.AluOpType.mult, mybir.AluOpType.min)
            return sc

        def input_chain(nsq_ps):
            """s_in = s1 * artanh(min(nx, MAX_NORM)) / nh  (faithful proj+logmap0)."""
            n2 = p_tmp.tile([128, BT], F32, tag="t0")
            nc.vector.tensor_scalar_max(n2, nsq_ps, EPS * EPS)
            nx = p_tmp.tile([128, BT], F32, tag="t1")
            nc.scalar.activation(nx, n2, AF.Sqrt)
            # nh = nx * min(1, MAX_NORM/nx) == min(nx, MAX_NORM)  (nx >= EPS > 0)
            nh = p_tmp.tile([128, BT], F32, tag="t2")
            nc.vector.tensor_scalar_min(nh, nx, MAX_NORM)
            onep = p_tmp.tile([128, BT], F32, tag="t3")
            nc.vector.tensor_scalar_add(onep, nh, 1.0)
            onem = p_tmp.tile([128, BT], F32, tag="t4")
            nc.vector.tensor_scalar(onem, nh, -1.0, 1.0, mybir.AluOpType.mult, mybir.AluOpType.add)
            rom = p_tmp.tile([128, BT], F32, tag="t5")
            nc.vector.reciprocal(rom, onem)
            ratio = p_tmp.tile([128, BT], F32, tag="t0")
            nc.vector.tensor_mul(ratio, onep, rom)
            lnr = p_tmp.tile([128, BT], F32, tag="t3")
            nc.scalar.activation(lnr, ratio, AF.Ln)  # = 2*artanh(nh)
            rnh = p_tmp.tile([128, BT], F32, tag="t4")
            nc.vector.reciprocal(rnh, nh)
            rnx = p_tmp.tile([128, BT], F32, tag="t5")
            nc.vector.reciprocal(rnx, nx)
            s1 = p_tmp.tile([128, BT], F32, tag="t0")
            nc.vector.tensor_scalar(s1, rnx, MAX_NORM, 1.0, mybir.AluOpType.mult, mybir.AluOpType.min)
            t1 = p_tmp.tile([128, BT], F32, tag="t2")
            nc.vector.tensor_mul(t1, lnr, rnh)
            t2 = p_tmp.tile([128, BT], F32, tag="t4")
            nc.vector.tensor_scalar_mul(t2, t1, 0.5)
            s_in = p_sc.tile([128, BT], F32)
            nc.vector.tensor_mul(s_in, t2, s1)
            return s_in

        n_groups = bpc // BT
        for g in range(n_groups):
            # ---- input stage: load, square, norms ----
            xs_list, adj_list = [], []
            nxsq = pp_n.tile([128, BT], F32, tag="nsq")
            for j in range(BT):
                b = g * BT + j
                xs = p_x.tile([128, D], F32R)
                nc.sync.dma_start(
                    out=xs.rearrange("p (c n) -> p c n", c=2),
                    in_=xT_d[b].rearrange("c p n -> p c n"),
                )
                adj_sb = p_adj.tile([128, N], F32)
                nc.sync.dma_start(out=adj_sb, in_=adjT_d[b])
                sqx = p_sq.tile([128, D], F32)
                nc.vector.tensor_mul(sqx, xs, xs)
                norm_mm(nxsq[:, j : j + 1], sqx)
                xs_list.append(xs)
                adj_list.append(adj_sb)
            sc_prev = input_chain(nxsq)
            cur = xs_list

            # ---- HGC layers ----
            for i in range(L):
                r_list = []
                nsq = pp_n.tile([128, BT], F32, tag="nsq")
                for j in range(BT):
                    u_ps = pp_u.tile([128, D], F32)
                    for c in range(2):
                        nc.tensor.matmul(
                            u_ps,
                            cur[j][:, c * 128 : (c + 1) * 128],
                            W_sb[:, (i * 2 + c) * D : (i * 2 + c + 1) * D],
                            start=(c == 0),
                            stop=(c == 1) and not has_bias,
                        )
                    if has_bias:
                        nc.tensor.matmul(
                            u_ps,
                            ones_row,
                            bs_sb[:, i * D : (i + 1) * D],
                            start=False,
                            stop=True,
                        )
                    u_sb = p_u.tile([128, D], F32)
                    nc.vector.tensor_scalar_mul(u_sb, u_ps, sc_prev[:, j : j + 1])
                    o2 = pp_o2.tile([128, D], F32)
                    for c in range(2):
                        nc.tensor.matmul(
                            o2[:, c * 128 : (c + 1) * 128],
                            u_sb[:, c * 128 : (c + 1) * 128],
                            adj_list[j],
                            start=True,
                            stop=True,
                        )
                    r = p_r.tile([128, D], F32R)
                    nc.scalar.activation(r, o2, AF.Relu)
                    sq = p_sq.tile([128, D], F32)
                    nc.vector.tensor_mul(sq, r, r)
                    norm_mm(nsq[:, j : j + 1], sq)
                    r_list.append(r)
                sc_prev = clip_chain(nsq)
                cur = r_list

            # ---- head ----
            for j in range(BT):
                b = g * BT + j
                h_ps = pp_h.tile([128, F], F32)
                for c in range(2):
                    nc.tensor.matmul(
                        h_ps,
                        cur[j][:, c * 128 : (c + 1) * 128],
                        Wout_sb[:, c * F : (c + 1) * F],
                        start=(c == 0),
                        stop=(c == 1) and not has_bout,
                    )
                if has_bout:
                    nc.tensor.matmul(h_ps, ones_row, bout_sb, start=False, stop=True)
                ho = p_out.tile([128, F], F32)
                nc.vector.tensor_scalar(
                    ho, h_ps, sc_prev[:, j : j + 1], mask_sb[:, b : b + 1],
                    mybir.AluOpType.mult, mybir.AluOpType.mult,
                )
                nc.sync.dma_start(out=out_d[b], in_=ho)

    nc.compile()  # bacc passes: split >1-wait instructions for TRN2 codegen
    return nc


_CACHE: dict = {}


def kernel(**inputs) -> np.ndarray:
    x = np.ascontiguousarray(np.asarray(inputs["x"], np.float32))
    adj = np.ascontiguousarray(np.asarray(inputs["adj"], np.float32))
    mask = np.ascontiguousarray(np.asarray(inputs["node_mask"], np.float32))
    Ws = np.ascontiguousarray(np.asarray(inputs["Ws"], np.float32))
    bs = np.asarray(inputs["bs"], np.float32)
    Wout = np.ascontiguousarray(np.asarray(inputs["Wout"], np.float32))
    bout = np.asarray(inputs["bout"], np.float32)

    has_bias = bool(np.any(bs))
    has_bout = bool(np.any(bout))
    key = (has_bias, has_bout)
    if key not in _CACHE:
        _CACHE[key] = _build(has_bias, has_bout)
    nc = _CACHE[key]

    # host-side relayouts: s-layout x (dim-major) and transposed adjacency
    xT = np.ascontiguousarray(x.transpose(0, 2, 1)).reshape(B, 2, 128, N)
    adjT = np.ascontiguousarray(adj.transpose(0, 2, 1))

    in_maps = []
    for c in range(NCORES):
        sl = slice(c * BPC, (c + 1) * BPC)
        m = {
            "xT": xT[sl],
            "adjT": adjT[sl],
            "mask": mask[sl],
            "Ws": Ws,
            "Wout": Wout,
        }
        if has_bias:
            m["bs"] = bs.reshape(L, 1, D)
        if has_bout:
            m["bout"] = bout.reshape(1, F)
        in_maps.append(m)

    res = run_bass_kernel_spmd(nc, in_maps, core_ids=list(range(NCORES)))
    out = np.concatenate([r["out"] for r in res.results], axis=0)
    return out.astype(np.float32)


if __name__ == "__main__":
    rng = np.random.default_rng(0)
    demo = {
        "x": 0.01 * rng.standard_normal((B, N, D), dtype=np.float32),
        "adj": rng.random((B, N, N), dtype=np.float32),
        "node_mask": np.ones((B, N, 1), np.float32),
        "Ws": rng.standard_normal((L, D, D), dtype=np.float32) / np.sqrt(D),
        "bs": np.zeros((L, D), np.float32),
        "Wout": rng.standard_normal((D, F), dtype=np.float32) / np.sqrt(D),
        "bout": np.zeros((F,), np.float32),
    }
    print(kernel(**demo).shape)
